# revision 29
# baseline (speedup 1.0000x reference)
"""Trainium2 kernel for per-class conditional dense (MoE-style routing).

    out[b] = x[b] @ W[classes[b]] + bias[classes[b]]
    x: [2048, 512] f32, classes: [2048, 1] int, W: [100, 512, 512] f32,
    bias: [100, 512] f32 -> out: [2048, 512] f32

Sharding: expert-parallel across 8 NeuronCores (grouped-GEMM style).
Class c is owned by core c // 13 (13 class slots per core). The host
routes each sample to the core owning its class, packing the samples of
each class into a fixed-width (S columns, zero-padded) block of a
transposed activation panel.

Precision: the weight table is the dominant HBM traffic, so it is
stored as TRN fp8 E3M4 (float8e3, 4 mantissa bits) at 1 byte/elem --
half the bf16 bytes. W is pre-scaled by a power of two s so its range
fills E3M4's [0.25, 15.5] normal band, and x is divided by the same s
(exact in bf16), so out = (x/s) @ (sW) = x @ W with no epilogue fixup.
The PE consumes the fp8 weights directly as the moving operand against
a bf16 stationary x panel (mixed-dtype matmul, 1 cycle/row). Measured
end-to-end relative error ~1.5e-2 (E3M4 quantization of W dominates),
inside the 2e-2 gate.

Data movement: ONE interleaved fp8 "mega" stream per core carries
everything in exact consumption order on a single Sync-HWDGE ring:
[x_slot0 | W_slot0 | x_slot1 | W_slot1 | ...], where each slot's bf16
x panel (transposed, k-tiled) rides as raw bytes inside the fp8 tensor
and is read back on-chip through a bitcast view. One ring = strict
FIFO arrival order (a slot's x always lands just before its weights),
2.3-6.9KB per-partition lines keep the 16 SDMA engines at their
per-packet service ceiling (~26 GB/s/engine), and no second ring
competes for packet slots during the PE-critical early phase.

The Tensor engine is the critical resource (~26.6K moving fp8 columns
= 11.1 us at 2.4 GHz): warm-up dummies bridge the DMA fill so the
DVFS p-state ramp (full clock only after ~3us of continuous matmuls,
reset by idle gaps) overlaps the wait; chunks ramp 0.5/0.5/1/1/2/3/3/
1/1 slots so the PE starts half a slot after first-byte and the stream
then stays just ahead; the final slot is split in U-halves on fresh
PSUM tiles so its first drain+store overlap its second half's matmuls
and the kernel-ending store is only 16 KB. PSUM groups (GRP slots
each) drain through Scalar-ACTIVATE / DVE alternately and store from
the Sync ring (whose FIFO defers store packets behind the remaining
weight stream for free). The host scatters the panel rows back to
sample order and adds the bias in fp32.
"""

import sys
import types

import numpy as np

try:
    import concourse.bass as bass
except ImportError:  # pragma: no cover - fallback for bare environments
    for _p in ("/opt/trn_rl_repo", "/root/.axon_site/_ro/trn_rl_repo"):
        if _p not in sys.path:
            sys.path.insert(0, _p)
    import concourse.bass as bass

try:  # pragma: no cover
    import antenv.axon_hooks  # noqa: F401
except ImportError:
    # bass_utils imports this when BASS_TRACE is set; the agent image's
    # antenv lacks it. Register a no-op shim so tracing degrades to a
    # plain (untraced) run instead of crashing.
    _hooks = types.ModuleType("antenv.axon_hooks")
    _hooks.get_axon_ntff_profile_hook = lambda: None
    _hooks.set_axon_ntff_profile_hook = lambda h: None
    sys.modules["antenv.axon_hooks"] = _hooks

import bass_rust
import ml_dtypes
import concourse.tile as tile
from concourse import mybir
from concourse.bass_utils import run_bass_kernel_spmd

B, D, U, C = 2048, 512, 512, 100
NCORES = 8
CPC = 13  # class slots per core (8 * 13 = 104 >= C)
PT = 128  # partition tile
KT = D // PT  # contraction-dim tiles
WSL = KT * U  # fp8 weight columns per slot
BF16 = ml_dtypes.bfloat16
FP8 = ml_dtypes.float8_e3m4
FP8_MAX = 15.5  # E3M4 max finite
N_WARM = 6  # 512-col PE warm-up matmuls (DVFS p-state ramp bridge)
N_WARM_FINE = 4  # trailing 128-col warm-up matmuls (finer granularity)

_PROG_CACHE = {}
LAST_RESULTS = None  # BassKernelResults of the most recent device run


def _split_multi_waits(nc):
    """Walrus on this image only accepts one sync wait per instruction.

    Tile emits multi-wait instructions (notably the kernel-tail Drain,
    which waits on every live semaphore). Split each extra wait onto a
    same-engine NoOp inserted immediately before the instruction.
    """
    for fn in nc.m.functions:
        for bb in fn.blocks:
            new = []
            changed = False
            for inst in bb.instructions:
                si = inst.sync_info
                waits = list(si.on_wait) if si else []
                if len(waits) > 1:
                    for idx, w in enumerate(waits[:-1]):
                        nop = mybir.InstNoOp(
                            name=f"{inst.name}-waitsplit{idx}", ins=[], outs=[]
                        )
                        nop.engine = inst.engine
                        nop.sync_info = bass_rust.SyncInfo(
                            on_wait=[w], on_update=[]
                        )
                        new.append(nop)
                    inst.sync_info = bass_rust.SyncInfo(
                        on_wait=[waits[-1]], on_update=list(si.on_update)
                    )
                    changed = True
                new.append(inst)
            if changed:
                bb.instructions = new


def _build_program(S):
    """One SPMD program, shared by all 8 cores; per-core data differs.

    Per core: mega [PT, CPC*(XBLK+WSL)] float8e3 -- per slot, the bf16
    x panel block (as raw bytes) followed by the slot's pre-scaled E3M4
    weights -- -> out [NCOL, U] bf16.
    """
    f32 = mybir.dt.float32
    bf16 = mybir.dt.bfloat16
    fp8 = mybir.dt.float8e3
    NCOL = CPC * S
    GRP = PT // S  # class slots sharing one PSUM bank / output tile
    OG = -(-CPC // GRP)  # output groups
    BR = GRP * S  # rows per PSUM bank / output tile
    XBLK = 2 * KT * S  # fp8 columns of the slot's bf16 x panel block
    SLOT = XBLK + WSL  # fp8 columns per slot block
    HW = WSL // 2

    nc = bass.Bass()
    mega = nc.dram_tensor("mega", [PT, CPC * SLOT], fp8, kind="ExternalInput")
    out = nc.dram_tensor("out", [NCOL, U], bf16, kind="ExternalOutput")

    # Chunk column widths: slot 0 in halves (x + first two k-tiles of
    # W, then the rest) so the PE starts ASAP, ramp to 3-slot chunks
    # (6.9KB lines, full packet-service rate), taper at the end.
    chunks = [XBLK + HW, HW]
    for n in (1, 1, 2, 3, 3, 1, 1):
        chunks.append(n * SLOT)
    assert sum(chunks) == CPC * SLOT

    with tile.TileContext(nc) as tc:
        with (
            tc.tile_pool(name="mp", bufs=1) as mp,
            tc.tile_pool(name="op", bufs=1) as op,
            tc.tile_pool(name="pp", bufs=3, space="PSUM") as pp,
            tc.tile_pool(name="pb", bufs=2, space="PSUM") as pbpool,
            tc.tile_pool(name="ap", bufs=1, space="PSUM") as apool,
        ):
            m_t = mp.tile([PT, CPC * SLOT], fp8, name="m")
            scr_s = mp.tile([PT, 512], bf16, name="scr")
            scr_p = apool.tile([PT, 512], f32, name="scrp")

            # Chunks alternate between the two HWDGE rings (Sync and
            # Scalar): the 16 SDMA engines round-robin between rings at
            # packet granularity, and two rings buffer ~40% more deeply
            # than one early in the stream. Each chunk lives wholly on
            # one ring, so per-chunk completion order still tracks the
            # slot order closely.
            col = 0
            for ci, w in enumerate(chunks):
                q = nc.sync if ci % 2 == 0 else nc.scalar
                q.dma_start(m_t[:, col : col + w], mega[:, col : col + w])
                col += w

            # PE warm-up: the DVFS p-state needs ~3us of continuous
            # matmul execution to reach full clock, and idle gaps reset
            # it (measured: cold matmuls run at ~1.2GHz, half speed).
            # Dummies on memset scratch bridge the DMA fill, 512-col
            # first then finer 128-col, ending near slot 0's arrival;
            # real matmuls continue the ramp seamlessly.
            nc.gpsimd.memset(scr_s[:], 1.0)
            for _ in range(N_WARM):
                nc.tensor.matmul(
                    scr_p[:],
                    scr_s[:, :PT],
                    scr_s[:],
                    start=True,
                    stop=True,
                    skip_group_check=True,
                )
            for _ in range(N_WARM_FINE):
                nc.tensor.matmul(
                    scr_p[:, :PT],
                    scr_s[:, :PT],
                    scr_s[:, :PT],
                    start=True,
                    stop=True,
                    skip_group_check=True,
                )

            ots = [op.tile([BR, U], bf16, name=f"o{g}") for g in range(OG)]

            def stat_ap(j, i):
                # The slot's bf16 x panel, k-tile i: raw bytes live at
                # fp8 columns [j*SLOT + i*2S, +2S); bitcast back.
                base = j * SLOT + i * 2 * S
                return m_t[:, base : base + 2 * S].bitcast(bf16)

            def mov_ap(j, i, lo=0, hi=U):
                base = j * SLOT + XBLK + i * U
                return m_t[:, base + lo : base + hi]

            U2 = U // 2
            for j in range(CPC):
                g, r = divmod(j, GRP)
                last = j == CPC - 1 and r == 0
                rows = min(BR, NCOL - g * BR)
                if not last:
                    if r == 0:
                        ps = pp.tile([BR, U], f32, tag="ps", name=f"ps{g}")
                    for i in range(KT):
                        nc.tensor.matmul(
                            ps[S * r : S * r + S, :],
                            stat_ap(j, i),
                            mov_ap(j, i),
                            start=(i == 0),
                            stop=(i == KT - 1),
                            # PE-array column offset = PSUM partition
                            # offset; auto-infer rejects some offsets,
                            # so pass it explicitly.
                            tile_position=(0, S * r),
                        )
                else:
                    # Final slot split in U-halves on fresh PSUM tiles
                    # (dep tracking is partition-granular; a shared
                    # tile would serialize half B behind half A's
                    # drain): half A's drain and store overlap half B's
                    # matmuls, and the kernel-ending store shrinks to
                    # 16 KB.
                    for uo in (0, U2):
                        psh = pbpool.tile(
                            [S, U2], f32, tag="psh", name=f"h{uo}"
                        )
                        for i in range(KT):
                            nc.tensor.matmul(
                                psh[:, :],
                                stat_ap(j, i),
                                mov_ap(j, i, uo, uo + U2),
                                start=(i == 0),
                                stop=(i == KT - 1),
                                tile_position=(0, 0),
                            )
                        nc.vector.tensor_scalar_add(
                            ots[g][:rows, uo : uo + U2], psh[:rows, :], 0.0
                        )
                        q = nc.sync if uo == 0 else nc.scalar
                        q.dma_start(
                            out[g * BR : g * BR + rows, uo : uo + U2],
                            ots[g][:rows, uo : uo + U2],
                        )
                if j == 0:
                    # Fillers covering the gap between slot 0's matmuls
                    # and the arrival of the next chunk; they read the
                    # already-arrived slot-0 x block (no new waits).
                    for _ in range(3):
                        nc.tensor.matmul(
                            scr_p[:S, :S],
                            stat_ap(0, 0),
                            stat_ap(0, 0),
                            start=True,
                            stop=True,
                            skip_group_check=True,
                        )
                if (r == GRP - 1 or j == CPC - 1) and not last:
                    # Drains alternate Scalar-ACTIVATE / DVE so
                    # consecutive groups' drains overlap at the tail.
                    if g % 2 == 0:
                        nc.scalar.copy(ots[g][:rows, :], ps[:rows, :])
                    else:
                        nc.vector.tensor_scalar_add(
                            ots[g][:rows, :], ps[:rows, :], 0.0
                        )
                    # Early stores ride the Sync ring: its FIFO
                    # naturally defers their packets behind the
                    # remaining weight stream (no mid-stream bandwidth
                    # theft). The second-to-last group's trigger goes
                    # on Scalar so the tail triggers issue concurrently
                    # (each costs ~0.6us).
                    q = nc.scalar if g == OG - 2 else nc.sync
                    q.dma_start(
                        out[g * BR : g * BR + rows, :], ots[g][:rows, :]
                    )
    _split_multi_waits(nc)
    return nc


def kernel(x, classes, kernel, bias):
    global LAST_RESULTS
    x = np.asarray(x, dtype=np.float32)
    W = np.asarray(kernel, dtype=np.float32)
    bias_np = np.asarray(bias, dtype=np.float32)
    cls = np.asarray(classes).reshape(-1).astype(np.int64)

    counts = np.bincount(cls, minlength=C)
    # Fixed column width per class slot; multiple of 8 for DMA alignment.
    S = int(max(32, -(-counts.max() // 8) * 8))
    if S not in _PROG_CACHE:
        _PROG_CACHE[S] = _build_program(S)
    nc = _PROG_CACHE[S]
    NCOL = CPC * S
    XBLK = 2 * KT * S
    SLOT = XBLK + WSL

    # Power-of-two weight scale filling E3M4's normal band; x carries
    # the inverse scale exactly (exponent shift), so out = x @ W.
    absmax = float(np.abs(W).max())
    s = float(2.0 ** np.floor(np.log2(FP8_MAX / absmax))) if absmax > 0 else 1.0

    order = np.argsort(cls, kind="stable")
    starts = np.zeros(C + 1, np.int64)
    np.cumsum(counts[:C], out=starts[1:])
    rows_by_class = [order[starts[c] : starts[c + 1]] for c in range(C)]

    # Weight slots, pre-tiled to the SBUF layout and cast to E3M4:
    # [c, p, i*U+u] holds s*W[c, i*128+p, u].
    W_t8 = (
        (W * s)
        .reshape(C, KT, PT, U)
        .transpose(0, 2, 1, 3)
        .reshape(C, PT, KT * U)
        .astype(FP8)
    )

    xs = x * np.float32(1.0 / s)
    in_maps = []
    for m in range(NCORES):
        xt_m = np.zeros((D, NCOL), np.float32)
        for j in range(CPC):
            c = m * CPC + j
            if c >= C:
                continue
            r = rows_by_class[c]
            if r.size:
                xt_m[:, S * j : S * j + r.size] = xs[r].T
        # Pre-tile x panel: [p, i*NCOL + c] = xt[i*128+p, c].
        xt_dev = np.ascontiguousarray(
            xt_m.reshape(KT, PT, NCOL).transpose(1, 0, 2).reshape(PT, KT * NCOL)
        ).astype(BF16)
        # Interleaved mega stream: per slot, the bf16 x block (raw
        # bytes viewed as fp8) then the slot's weights.
        mega_m = np.empty((PT, CPC * SLOT), FP8)
        for j in range(CPC):
            c = (m * CPC + j) % C
            xb = np.ascontiguousarray(
                np.concatenate(
                    [
                        xt_dev[:, i * NCOL + S * j : i * NCOL + S * (j + 1)]
                        for i in range(KT)
                    ],
                    axis=1,
                )
            ).view(FP8)
            mega_m[:, j * SLOT : j * SLOT + XBLK] = xb
            mega_m[:, j * SLOT + XBLK : (j + 1) * SLOT] = W_t8[c]
        in_maps.append({"mega": mega_m})

    res = run_bass_kernel_spmd(nc, in_maps, list(range(NCORES)))
    LAST_RESULTS = res

    out = np.empty((B, U), np.float32)
    for m in range(NCORES):
        panel = np.asarray(res.results[m]["out"]).astype(np.float32)
        for j in range(CPC):
            c = m * CPC + j
            if c >= C:
                continue
            r = rows_by_class[c]
            if r.size:
                out[r] = panel[S * j : S * j + r.size] + bias_np[c]
    return out


# revision 30
# speedup vs baseline: 1.0407x; 1.0407x over previous
"""Trainium2 kernel for per-class conditional dense (MoE-style routing).

    out[b] = x[b] @ W[classes[b]] + bias[classes[b]]
    x: [2048, 512] f32, classes: [2048, 1] int, W: [100, 512, 512] f32,
    bias: [100, 512] f32 -> out: [2048, 512] f32

Sharding: expert-parallel across 8 NeuronCores (grouped-GEMM style).
Class c is owned by core c // 13 (13 class slots per core). The host
routes each sample to the core owning its class, packing the samples of
each class into a fixed-width (S columns, zero-padded) block of a
transposed activation panel.

Precision strategy: the weight table is the dominant HBM traffic, so it
is stored as TRN fp8 E3M4 (float8e3, 4 mantissa bits) at 1 byte/elem --
half the bf16 bytes. W is pre-scaled by a power of two s so its range
fills E3M4's [0.25, 15.5] normal band, and x is divided by the same s
(exact in bf16), so out = (x/s) @ (sW) = x @ W with no epilogue fixup.
The PE consumes the fp8 weights directly as the moving operand against
a bf16 stationary x panel (mixed-dtype matmul, 1 cycle/row). Measured
end-to-end relative error ~1.5e-2 (E3M4 quantization of W dominates),
inside the 2e-2 gate.

With the stream halved the Tensor engine becomes the critical resource
(~26.6K moving columns ~= 11.1 us at 2.4 GHz), so the schedule is built
around keeping the PE hot: warm-up dummy matmuls run during the DMA
fill so the PE's DVFS p-state is fully ramped when real data lands; the
weight stream arrives in ramped chunks (1,1,1,2,2,3,3 slots) on the
Sync HWDGE queue so early slots land ASAP and the stream then stays
ahead of the PE; the x panel rides the Scalar HWDGE queue in two
k-halves so the first matmuls are gated only by their own bytes.
PSUM groups (GRP slots each) drain through Scalar ACTIVATEs into bf16
tiles and are stored from the Scalar queue right away -- no
cross-engine waits anywhere in the drain path. The host scatters the
panel rows back to sample order and adds the bias in fp32.
"""

import sys
import types

import numpy as np

try:
    import concourse.bass as bass
except ImportError:  # pragma: no cover - fallback for bare environments
    for _p in ("/opt/trn_rl_repo", "/root/.axon_site/_ro/trn_rl_repo"):
        if _p not in sys.path:
            sys.path.insert(0, _p)
    import concourse.bass as bass

try:  # pragma: no cover
    import antenv.axon_hooks  # noqa: F401
except ImportError:
    # bass_utils imports this when BASS_TRACE is set; the agent image's
    # antenv lacks it. Register a no-op shim so tracing degrades to a
    # plain (untraced) run instead of crashing.
    _hooks = types.ModuleType("antenv.axon_hooks")
    _hooks.get_axon_ntff_profile_hook = lambda: None
    _hooks.set_axon_ntff_profile_hook = lambda h: None
    sys.modules["antenv.axon_hooks"] = _hooks

import bass_rust
import ml_dtypes
import concourse.tile as tile
from concourse import mybir
from concourse.bass_utils import run_bass_kernel_spmd

B, D, U, C = 2048, 512, 512, 100
NCORES = 8
CPC = 13  # class slots per core (8 * 13 = 104 >= C)
PT = 128  # partition tile
KT = D // PT  # contraction-dim tiles
WSL = KT * U  # fp8 weight columns per slot
BF16 = ml_dtypes.bfloat16
FP8 = ml_dtypes.float8_e3m4
FP8_MAX = 15.5  # E3M4 max finite
N_WARM = 6  # 512-col no-dep PE warm-up matmuls (DVFS p-state ramp)
N_WARM_FINE = 4  # trailing 128-col warm-up matmuls (fine-grained bridge)
N_WARM_POST = 10  # 128-col xt0-gated matmuls bridging to the first chunk

# Weight-chunk ramp (in half-slot units): slot 0 arrives in k-halves
# so the PE's first matmuls start half a slot earlier, mid-stream
# chunks grow to 3 slots whose 6KB-per-partition lines sustain the
# full packet-service rate (~26 GB/s/engine measured vs ~19 for 2KB
# lines), then a taper so the final slots' matmuls trail the last HBM
# byte minimally.
CHUNK_HALVES = [1, 1, 2, 2, 4, 6, 6, 2, 2]
assert sum(CHUNK_HALVES) == 2 * CPC
HWSL = WSL // 2  # fp8 columns per half-slot

_PROG_CACHE = {}
LAST_RESULTS = None  # BassKernelResults of the most recent device run


def _split_multi_waits(nc):
    """Walrus on this image only accepts one sync wait per instruction.

    Tile emits multi-wait instructions (notably the kernel-tail Drain,
    which waits on every live semaphore). Split each extra wait onto a
    same-engine NoOp inserted immediately before the instruction.
    """
    for fn in nc.m.functions:
        for bb in fn.blocks:
            new = []
            changed = False
            for inst in bb.instructions:
                si = inst.sync_info
                waits = list(si.on_wait) if si else []
                if len(waits) > 1:
                    for idx, w in enumerate(waits[:-1]):
                        nop = mybir.InstNoOp(
                            name=f"{inst.name}-waitsplit{idx}", ins=[], outs=[]
                        )
                        nop.engine = inst.engine
                        nop.sync_info = bass_rust.SyncInfo(
                            on_wait=[w], on_update=[]
                        )
                        new.append(nop)
                    inst.sync_info = bass_rust.SyncInfo(
                        on_wait=[waits[-1]], on_update=list(si.on_update)
                    )
                    changed = True
                new.append(inst)
            if changed:
                bb.instructions = new


def _build_program(S):
    """One SPMD program, shared by all 8 cores; per-core data differs.

    Per core: xt [PT, KT*NCOL] bf16 (pre-tiled transposed class-blocked
    activations, pre-divided by the weight scale), wt [PT, CPC*WSL]
    float8e3 (pre-tiled, pre-scaled weight slots, slot-major columns)
    -> out [NCOL, U] bf16.
    """
    f32 = mybir.dt.float32
    bf16 = mybir.dt.bfloat16
    fp8 = mybir.dt.float8e3
    NCOL = CPC * S
    GRP = PT // S  # class slots sharing one PSUM bank / output tile
    OG = -(-CPC // GRP)  # output groups
    BR = GRP * S  # rows per PSUM bank / output tile

    nc = bass.Bass()
    xt = nc.dram_tensor("xt", [PT, KT * NCOL], bf16, kind="ExternalInput")
    # Slot 0's x panel, duplicated into its own tiny contiguous tensor
    # (32 KB): it lands ~1us before the full panel, so the PE's first
    # real matmuls are gated only by slot 0's weights.
    xt0 = nc.dram_tensor("xt0", [PT, KT * S], bf16, kind="ExternalInput")
    wt = nc.dram_tensor("wt", [PT, CPC * WSL], fp8, kind="ExternalInput")
    out = nc.dram_tensor("out", [NCOL, U], bf16, kind="ExternalOutput")

    with tile.TileContext(nc) as tc:
        with (
            tc.tile_pool(name="xp", bufs=1) as xp,
            tc.tile_pool(name="wp", bufs=1) as wp,
            tc.tile_pool(name="op", bufs=1) as op,
            tc.tile_pool(name="pp", bufs=3, space="PSUM") as pp,
            tc.tile_pool(name="pb", bufs=2, space="PSUM") as pbpool,
            tc.tile_pool(name="ap", bufs=1, space="PSUM") as apool,
        ):
            xt_t = xp.tile([PT, KT * NCOL], bf16, name="x")
            xt0_t = xp.tile([PT, KT * S], bf16, name="x0")
            w_t = wp.tile([PT, CPC * WSL], fp8, name="w")
            scr_s = xp.tile([PT, 512], bf16, name="scr")
            scr_p = apool.tile([PT, 512], f32, name="scrp")

            # Scalar HWDGE ring: slot-0 x panel first (tiny), then the
            # full panel as ONE transfer (3.3KB lines pack better than
            # two 1.6KB halves). Sync HWDGE ring: the weight chunks, in
            # slot order. Two rings share the 16 SDMA engines but
            # buffer independently.
            nc.scalar.dma_start(xt0_t[:], xt0[:, :])
            nc.scalar.dma_start(xt_t[:], xt[:, :])
            col = 0
            for n in CHUNK_HALVES:
                w = n * HWSL
                nc.sync.dma_start(w_t[:, col : col + w], wt[:, col : col + w])
                col += w

            # PE warm-up: the DVFS p-state needs ~3us of continuous
            # matmul execution to reach full clock, and any multi-us
            # idle gap resets it (measured: cold matmuls run at ~1.2GHz,
            # half speed). Free-running dummies on memset scratch
            # bridge the DMA fill; one dummy then absorbs the xt0 DMA
            # wait (LDWEIGHTS carries a single wait), and xt0-gated
            # 128-col dummies keep the PE hot until slot 0's weights
            # land.
            nc.gpsimd.memset(scr_s[:], 1.0)
            for _ in range(N_WARM):
                nc.tensor.matmul(
                    scr_p[:],
                    scr_s[:, :PT],
                    scr_s[:],
                    start=True,
                    stop=True,
                    skip_group_check=True,
                )
            for _ in range(N_WARM_FINE):
                nc.tensor.matmul(
                    scr_p[:, :PT],
                    scr_s[:, :PT],
                    scr_s[:, :PT],
                    start=True,
                    stop=True,
                    skip_group_check=True,
                )
            for _ in range(N_WARM_POST):
                nc.tensor.matmul(
                    scr_p[:, :PT],
                    xt0_t[:, :PT],
                    xt0_t[:, :PT],
                    start=True,
                    stop=True,
                    skip_group_check=True,
                )

            ots = [op.tile([BR, U], bf16, name=f"o{g}") for g in range(OG)]

            U2 = U // 2
            for j in range(CPC):
                g, r = divmod(j, GRP)
                last = j == CPC - 1 and r == 0
                if r == 0 and not last:
                    ps = pp.tile([BR, U], f32, tag="ps", name=f"ps{g}")
                rows = min(BR, NCOL - g * BR)
                if not last:
                    for i in range(KT):
                        if j == 0:
                            stat = xt0_t[:, i * S : (i + 1) * S]
                        else:
                            stat = xt_t[
                                :, i * NCOL + S * j : i * NCOL + S * (j + 1)
                            ]
                        nc.tensor.matmul(
                            ps[S * r : S * r + S, :],
                            stat,
                            w_t[:, (j * KT + i) * U : (j * KT + i + 1) * U],
                            start=(i == 0),
                            stop=(i == KT - 1),
                            # PE-array column offset = PSUM partition
                            # offset; auto-infer rejects some offsets,
                            # so pass it explicitly.
                            tile_position=(0, S * r),
                        )
                else:
                    # Final slot split in U-halves on fresh PSUM tiles
                    # (dep tracking is partition-granular; a shared
                    # tile would serialize half B behind half A's
                    # drain): half A's drain and store overlap half B's
                    # matmuls, and the kernel-ending store shrinks to
                    # 16 KB.
                    for uo in (0, U2):
                        psh = pbpool.tile([S, U2], f32, tag="psh", name=f"h{uo}")
                        for i in range(KT):
                            stat = xt_t[
                                :, i * NCOL + S * j : i * NCOL + S * (j + 1)
                            ]
                            nc.tensor.matmul(
                                psh[:, :],
                                stat,
                                w_t[
                                    :,
                                    (j * KT + i) * U + uo : (j * KT + i) * U
                                    + uo
                                    + U2,
                                ],
                                start=(i == 0),
                                stop=(i == KT - 1),
                                tile_position=(0, 0),
                            )
                        nc.vector.tensor_scalar_add(
                            ots[g][:rows, uo : uo + U2], psh[:rows, :], 0.0
                        )
                        q = nc.sync if uo == 0 else nc.scalar
                        q.dma_start(
                            out[g * BR : g * BR + rows, uo : uo + U2],
                            ots[g][:rows, uo : uo + U2],
                        )
                if j == 0:
                    # Fillers covering the gap between slot 0's matmuls
                    # and the arrival of the next weight chunk.
                    for _ in range(3):
                        nc.tensor.matmul(
                            scr_p[:, :PT],
                            xt0_t[:, :PT],
                            xt0_t[:, :PT],
                            start=True,
                            stop=True,
                            skip_group_check=True,
                        )
                if (r == GRP - 1 or j == CPC - 1) and not last:
                    # Drains alternate Scalar-ACTIVATE / DVE so
                    # consecutive groups' drains overlap at the tail.
                    if g % 2 == 0:
                        nc.scalar.copy(ots[g][:rows, :], ps[:rows, :])
                    else:
                        nc.vector.tensor_scalar_add(
                            ots[g][:rows, :], ps[:rows, :], 0.0
                        )
                    # Early stores ride the Sync ring: its FIFO
                    # naturally defers their packets behind the
                    # remaining weight stream (no mid-stream bandwidth
                    # theft). The second-to-last group's trigger goes
                    # on Scalar so the tail triggers issue concurrently
                    # (each costs ~0.6us).
                    q = nc.scalar if g == OG - 2 else nc.sync
                    q.dma_start(
                        out[g * BR : g * BR + rows, :], ots[g][:rows, :]
                    )
    _split_multi_waits(nc)
    return nc


def kernel(x, classes, kernel, bias):
    global LAST_RESULTS
    x = np.asarray(x, dtype=np.float32)
    W = np.asarray(kernel, dtype=np.float32)
    bias_np = np.asarray(bias, dtype=np.float32)
    cls = np.asarray(classes).reshape(-1).astype(np.int64)

    counts = np.bincount(cls, minlength=C)
    # Fixed column width per class slot; multiple of 8 for DMA alignment.
    S = int(max(32, -(-counts.max() // 8) * 8))
    if S not in _PROG_CACHE:
        _PROG_CACHE[S] = _build_program(S)
    nc = _PROG_CACHE[S]
    NCOL = CPC * S

    # Power-of-two weight scale filling E3M4's normal band; x carries
    # the inverse scale exactly (exponent shift), so out = x @ W.
    absmax = float(np.abs(W).max())
    s = float(2.0 ** np.floor(np.log2(FP8_MAX / absmax))) if absmax > 0 else 1.0

    order = np.argsort(cls, kind="stable")
    starts = np.zeros(C + 1, np.int64)
    np.cumsum(counts[:C], out=starts[1:])
    rows_by_class = [order[starts[c] : starts[c + 1]] for c in range(C)]

    # Weight slots, pre-tiled to the SBUF layout and cast to E3M4:
    # [c, p, i*U+u] holds s*W[c, i*128+p, u] so each slot's DMA line is
    # 2 KB contiguous per partition.
    W_t8 = (
        (W * s)
        .reshape(C, KT, PT, U)
        .transpose(0, 2, 1, 3)
        .reshape(C, PT, KT * U)
        .astype(FP8)
    )

    xs = x * np.float32(1.0 / s)
    in_maps = []
    for m in range(NCORES):
        xt_m = np.zeros((D, NCOL), np.float32)
        for j in range(CPC):
            c = m * CPC + j
            if c >= C:
                continue
            r = rows_by_class[c]
            if r.size:
                xt_m[:, S * j : S * j + r.size] = xs[r].T
        # Pre-tile x panel: [p, i*NCOL + c] = xt[i*128+p, c].
        xt_dev = np.ascontiguousarray(
            xt_m.reshape(KT, PT, NCOL).transpose(1, 0, 2).reshape(PT, KT * NCOL)
        ).astype(BF16)
        # Slot 0's panel duplicated contiguously: [p, i*S + c].
        xt0_dev = np.ascontiguousarray(
            np.concatenate(
                [xt_dev[:, i * NCOL : i * NCOL + S] for i in range(KT)], axis=1
            )
        )
        # Per-core weight panel: 13 slots concatenated slot-major along
        # the column axis (wrap-around classes pad the last core).
        slot_cls = [(m * CPC + j) % C for j in range(CPC)]
        wt_m = np.ascontiguousarray(
            np.concatenate([W_t8[c] for c in slot_cls], axis=1)
        )
        in_maps.append({"xt": xt_dev, "xt0": xt0_dev, "wt": wt_m})

    res = run_bass_kernel_spmd(nc, in_maps, list(range(NCORES)))
    LAST_RESULTS = res

    out = np.empty((B, U), np.float32)
    for m in range(NCORES):
        panel = np.asarray(res.results[m]["out"]).astype(np.float32)
        for j in range(CPC):
            c = m * CPC + j
            if c >= C:
                continue
            r = rows_by_class[c]
            if r.size:
                out[r] = panel[S * j : S * j + r.size] + bias_np[c]
    return out


# revision 31
# speedup vs baseline: 1.0727x; 1.0307x over previous
"""Trainium2 kernel for per-class conditional dense (MoE-style routing).

    out[b] = x[b] @ W[classes[b]] + bias[classes[b]]
    x: [2048, 512] f32, classes: [2048, 1] int, W: [100, 512, 512] f32,
    bias: [100, 512] f32 -> out: [2048, 512] f32

Sharding: expert-parallel across 8 NeuronCores (grouped-GEMM style).
Class c is owned by core c // 13 (13 class slots per core). The host
routes each sample to the core owning its class, packing the samples of
each class into a fixed-width (S columns, zero-padded) block of a
transposed activation panel.

Precision strategy: the weight table is the dominant HBM traffic, so it
is stored as TRN fp8 E3M4 (float8e3, 4 mantissa bits) at 1 byte/elem --
half the bf16 bytes. W is pre-scaled by a power of two s so its range
fills E3M4's [0.25, 15.5] normal band, and x is divided by the same s
(exact in bf16), so out = (x/s) @ (sW) = x @ W with no epilogue fixup.
The PE consumes the fp8 weights directly as the moving operand against
a bf16 stationary x panel (mixed-dtype matmul, 1 cycle/row). Measured
end-to-end relative error ~1.5e-2 (E3M4 quantization of W dominates),
inside the 2e-2 gate.

With the stream halved the Tensor engine becomes the critical resource
(~26.6K moving columns ~= 11.1 us at 2.4 GHz), so the schedule keeps
the PE hot and the 16 SDMA engines at their per-packet service ceiling:

- Sync HWDGE ring: the weight chunks in slot order, ramping
  0.5/0.5/1/1/2/3/3/1/1 slots -- small first so the PE starts half a
  slot after first-byte, 6KB-per-partition lines mid-stream (~26
  GB/s/engine vs ~19 for 2KB lines), tapered at the end so the final
  slots' matmuls trail the last HBM byte minimally.
- Scalar HWDGE ring: slot 0's x panel duplicated into a tiny tensor
  (lands ~1us before the full panel, so the first matmuls are gated
  only by slot 0's weights), then the full x panel as one transfer.
- PE warm-up: the DVFS p-state reaches full clock only after ~3us of
  continuous matmul execution and resets on idle gaps (cold matmuls
  measure ~1.2GHz, half speed), so dummy matmuls on scratch bridge the
  DMA fill and the real matmuls continue the ramp seamlessly.
- Drains alternate Scalar-ACTIVATE / DVE so consecutive groups
  overlap; stores ride the Sync ring, whose FIFO defers their packets
  behind the remaining weight stream for free; the final slot is
  computed in U-halves on fresh PSUM tiles so its first drain+store
  overlap its second half's matmuls and the kernel-ending store is
  only 16 KB (post-stream stores are per-packet-service-bound, and the
  last store's HBM write receipt is on the critical path).

The host scatters the panel rows back to sample order and adds the
bias in fp32. Measured 26.6-29.4 us across runs (DVFS throttle and
thermal state dominate the spread) vs ~35 us for the bf16 baseline.
"""

import sys
import types

import numpy as np

try:
    import concourse.bass as bass
except ImportError:  # pragma: no cover - fallback for bare environments
    for _p in ("/opt/trn_rl_repo", "/root/.axon_site/_ro/trn_rl_repo"):
        if _p not in sys.path:
            sys.path.insert(0, _p)
    import concourse.bass as bass

try:  # pragma: no cover
    import antenv.axon_hooks  # noqa: F401
except ImportError:
    # bass_utils imports this when BASS_TRACE is set; the agent image's
    # antenv lacks it. Register a no-op shim so tracing degrades to a
    # plain (untraced) run instead of crashing.
    _hooks = types.ModuleType("antenv.axon_hooks")
    _hooks.get_axon_ntff_profile_hook = lambda: None
    _hooks.set_axon_ntff_profile_hook = lambda h: None
    sys.modules["antenv.axon_hooks"] = _hooks

import bass_rust
import ml_dtypes
import concourse.tile as tile
from concourse import mybir
from concourse.bass_utils import run_bass_kernel_spmd

B, D, U, C = 2048, 512, 512, 100
NCORES = 8
CPC = 13  # class slots per core (8 * 13 = 104 >= C)
PT = 128  # partition tile
KT = D // PT  # contraction-dim tiles
WSL = KT * U  # fp8 weight columns per slot
BF16 = ml_dtypes.bfloat16
FP8 = ml_dtypes.float8_e3m4
FP8_MAX = 15.5  # E3M4 max finite
N_WARM = 6  # 512-col no-dep PE warm-up matmuls (DVFS p-state ramp)
N_WARM_FINE = 4  # trailing 128-col warm-up matmuls (fine-grained bridge)
N_WARM_POST = 10  # 128-col xt0-gated matmuls bridging to the first chunk

# Weight-chunk ramp (in half-slot units): slot 0 arrives in k-halves
# so the PE's first matmuls start half a slot earlier, mid-stream
# chunks grow to 3 slots whose 6KB-per-partition lines sustain the
# full packet-service rate (~26 GB/s/engine measured vs ~19 for 2KB
# lines), then a taper so the final slots' matmuls trail the last HBM
# byte minimally.
CHUNK_HALVES = [1, 1, 2, 2, 4, 6, 6, 2, 2]
assert sum(CHUNK_HALVES) == 2 * CPC
HWSL = WSL // 2  # fp8 columns per half-slot

_PROG_CACHE = {}
LAST_RESULTS = None  # BassKernelResults of the most recent device run


def _split_multi_waits(nc):
    """Walrus on this image only accepts one sync wait per instruction.

    Tile emits multi-wait instructions (notably the kernel-tail Drain,
    which waits on every live semaphore). Split each extra wait onto a
    same-engine NoOp inserted immediately before the instruction.
    """
    for fn in nc.m.functions:
        for bb in fn.blocks:
            new = []
            changed = False
            for inst in bb.instructions:
                si = inst.sync_info
                waits = list(si.on_wait) if si else []
                if len(waits) > 1:
                    for idx, w in enumerate(waits[:-1]):
                        nop = mybir.InstNoOp(
                            name=f"{inst.name}-waitsplit{idx}", ins=[], outs=[]
                        )
                        nop.engine = inst.engine
                        nop.sync_info = bass_rust.SyncInfo(
                            on_wait=[w], on_update=[]
                        )
                        new.append(nop)
                    inst.sync_info = bass_rust.SyncInfo(
                        on_wait=[waits[-1]], on_update=list(si.on_update)
                    )
                    changed = True
                new.append(inst)
            if changed:
                bb.instructions = new


def _build_program(S):
    """One SPMD program, shared by all 8 cores; per-core data differs.

    Per core: xt [PT, KT*NCOL] bf16 (pre-tiled transposed class-blocked
    activations, pre-divided by the weight scale), wt [PT, CPC*WSL]
    float8e3 (pre-tiled, pre-scaled weight slots, slot-major columns)
    -> out [NCOL, U] bf16.
    """
    f32 = mybir.dt.float32
    bf16 = mybir.dt.bfloat16
    fp8 = mybir.dt.float8e3
    NCOL = CPC * S
    GRP = PT // S  # class slots sharing one PSUM bank / output tile
    OG = -(-CPC // GRP)  # output groups
    BR = GRP * S  # rows per PSUM bank / output tile

    nc = bass.Bass()
    xt = nc.dram_tensor("xt", [PT, KT * NCOL], bf16, kind="ExternalInput")
    # Slot 0's x panel, duplicated into its own tiny contiguous tensor
    # (32 KB): it lands ~1us before the full panel, so the PE's first
    # real matmuls are gated only by slot 0's weights.
    xt0 = nc.dram_tensor("xt0", [PT, KT * S], bf16, kind="ExternalInput")
    wt = nc.dram_tensor("wt", [PT, CPC * WSL], fp8, kind="ExternalInput")
    out = nc.dram_tensor("out", [NCOL, U], bf16, kind="ExternalOutput")

    with tile.TileContext(nc) as tc:
        with (
            tc.tile_pool(name="xp", bufs=1) as xp,
            tc.tile_pool(name="wp", bufs=1) as wp,
            tc.tile_pool(name="op", bufs=1) as op,
            tc.tile_pool(name="pp", bufs=3, space="PSUM") as pp,
            tc.tile_pool(name="pb", bufs=2, space="PSUM") as pbpool,
            tc.tile_pool(name="ap", bufs=1, space="PSUM") as apool,
        ):
            xt_t = xp.tile([PT, KT * NCOL], bf16, name="x")
            xt0_t = xp.tile([PT, KT * S], bf16, name="x0")
            w_t = wp.tile([PT, CPC * WSL], fp8, name="w")
            scr_s = xp.tile([PT, 512], bf16, name="scr")
            scr_p = apool.tile([PT, 512], f32, name="scrp")

            # Scalar HWDGE ring: slot-0 x panel first (tiny), then the
            # full panel as ONE transfer (3.3KB lines pack better than
            # two 1.6KB halves). Sync HWDGE ring: the weight chunks, in
            # slot order. Two rings share the 16 SDMA engines but
            # buffer independently.
            nc.scalar.dma_start(xt0_t[:], xt0[:, :])
            nc.scalar.dma_start(xt_t[:], xt[:, :])
            col = 0
            for n in CHUNK_HALVES:
                w = n * HWSL
                nc.sync.dma_start(w_t[:, col : col + w], wt[:, col : col + w])
                col += w

            # PE warm-up: the DVFS p-state needs ~3us of continuous
            # matmul execution to reach full clock, and any multi-us
            # idle gap resets it (measured: cold matmuls run at ~1.2GHz,
            # half speed). Free-running dummies on memset scratch
            # bridge the DMA fill; one dummy then absorbs the xt0 DMA
            # wait (LDWEIGHTS carries a single wait), and xt0-gated
            # 128-col dummies keep the PE hot until slot 0's weights
            # land.
            nc.gpsimd.memset(scr_s[:], 1.0)
            for _ in range(N_WARM):
                nc.tensor.matmul(
                    scr_p[:],
                    scr_s[:, :PT],
                    scr_s[:],
                    start=True,
                    stop=True,
                    skip_group_check=True,
                )
            for _ in range(N_WARM_FINE):
                nc.tensor.matmul(
                    scr_p[:, :PT],
                    scr_s[:, :PT],
                    scr_s[:, :PT],
                    start=True,
                    stop=True,
                    skip_group_check=True,
                )
            for _ in range(N_WARM_POST):
                nc.tensor.matmul(
                    scr_p[:, :PT],
                    xt0_t[:, :PT],
                    xt0_t[:, :PT],
                    start=True,
                    stop=True,
                    skip_group_check=True,
                )

            ots = [op.tile([BR, U], bf16, name=f"o{g}") for g in range(OG)]

            U2 = U // 2
            for j in range(CPC):
                g, r = divmod(j, GRP)
                last = j == CPC - 1 and r == 0
                if r == 0 and not last:
                    ps = pp.tile([BR, U], f32, tag="ps", name=f"ps{g}")
                rows = min(BR, NCOL - g * BR)
                if not last:
                    for i in range(KT):
                        if j == 0:
                            stat = xt0_t[:, i * S : (i + 1) * S]
                        else:
                            stat = xt_t[
                                :, i * NCOL + S * j : i * NCOL + S * (j + 1)
                            ]
                        nc.tensor.matmul(
                            ps[S * r : S * r + S, :],
                            stat,
                            w_t[:, (j * KT + i) * U : (j * KT + i + 1) * U],
                            start=(i == 0),
                            stop=(i == KT - 1),
                            # PE-array column offset = PSUM partition
                            # offset; auto-infer rejects some offsets,
                            # so pass it explicitly.
                            tile_position=(0, S * r),
                        )
                else:
                    # Final slot split in U-halves on fresh PSUM tiles
                    # (dep tracking is partition-granular; a shared
                    # tile would serialize half B behind half A's
                    # drain): half A's drain and store overlap half B's
                    # matmuls, and the kernel-ending store shrinks to
                    # 16 KB.
                    for uo in (0, U2):
                        psh = pbpool.tile([S, U2], f32, tag="psh", name=f"h{uo}")
                        for i in range(KT):
                            stat = xt_t[
                                :, i * NCOL + S * j : i * NCOL + S * (j + 1)
                            ]
                            nc.tensor.matmul(
                                psh[:, :],
                                stat,
                                w_t[
                                    :,
                                    (j * KT + i) * U + uo : (j * KT + i) * U
                                    + uo
                                    + U2,
                                ],
                                start=(i == 0),
                                stop=(i == KT - 1),
                                tile_position=(0, 0),
                            )
                        nc.vector.tensor_scalar_add(
                            ots[g][:rows, uo : uo + U2], psh[:rows, :], 0.0
                        )
                        q = nc.sync if uo == 0 else nc.scalar
                        q.dma_start(
                            out[g * BR : g * BR + rows, uo : uo + U2],
                            ots[g][:rows, uo : uo + U2],
                        )
                if j == 0:
                    # Fillers covering the gap between slot 0's matmuls
                    # and the arrival of the next weight chunk.
                    for _ in range(3):
                        nc.tensor.matmul(
                            scr_p[:, :PT],
                            xt0_t[:, :PT],
                            xt0_t[:, :PT],
                            start=True,
                            stop=True,
                            skip_group_check=True,
                        )
                if (r == GRP - 1 or j == CPC - 1) and not last:
                    # Drains alternate Scalar-ACTIVATE / DVE so
                    # consecutive groups' drains overlap at the tail.
                    if g % 2 == 0:
                        nc.scalar.copy(ots[g][:rows, :], ps[:rows, :])
                    else:
                        nc.vector.tensor_scalar_add(
                            ots[g][:rows, :], ps[:rows, :], 0.0
                        )
                    # Early stores ride the Sync ring: its FIFO
                    # naturally defers their packets behind the
                    # remaining weight stream (no mid-stream bandwidth
                    # theft). The second-to-last group's trigger goes
                    # on Scalar so the tail triggers issue concurrently
                    # (each costs ~0.6us).
                    q = nc.scalar if g == OG - 2 else nc.sync
                    q.dma_start(
                        out[g * BR : g * BR + rows, :], ots[g][:rows, :]
                    )
    _split_multi_waits(nc)
    return nc


def kernel(x, classes, kernel, bias):
    global LAST_RESULTS
    x = np.asarray(x, dtype=np.float32)
    W = np.asarray(kernel, dtype=np.float32)
    bias_np = np.asarray(bias, dtype=np.float32)
    cls = np.asarray(classes).reshape(-1).astype(np.int64)

    counts = np.bincount(cls, minlength=C)
    # Fixed column width per class slot; multiple of 8 for DMA alignment.
    S = int(max(32, -(-counts.max() // 8) * 8))
    if S not in _PROG_CACHE:
        _PROG_CACHE[S] = _build_program(S)
    nc = _PROG_CACHE[S]
    NCOL = CPC * S

    # Power-of-two weight scale filling E3M4's normal band; x carries
    # the inverse scale exactly (exponent shift), so out = x @ W.
    absmax = float(np.abs(W).max())
    s = float(2.0 ** np.floor(np.log2(FP8_MAX / absmax))) if absmax > 0 else 1.0

    order = np.argsort(cls, kind="stable")
    starts = np.zeros(C + 1, np.int64)
    np.cumsum(counts[:C], out=starts[1:])
    rows_by_class = [order[starts[c] : starts[c + 1]] for c in range(C)]

    # Weight slots, pre-tiled to the SBUF layout and cast to E3M4:
    # [c, p, i*U+u] holds s*W[c, i*128+p, u] so each slot's DMA line is
    # 2 KB contiguous per partition.
    W_t8 = (
        (W * s)
        .reshape(C, KT, PT, U)
        .transpose(0, 2, 1, 3)
        .reshape(C, PT, KT * U)
        .astype(FP8)
    )

    xs = x * np.float32(1.0 / s)
    in_maps = []
    for m in range(NCORES):
        xt_m = np.zeros((D, NCOL), np.float32)
        for j in range(CPC):
            c = m * CPC + j
            if c >= C:
                continue
            r = rows_by_class[c]
            if r.size:
                xt_m[:, S * j : S * j + r.size] = xs[r].T
        # Pre-tile x panel: [p, i*NCOL + c] = xt[i*128+p, c].
        xt_dev = np.ascontiguousarray(
            xt_m.reshape(KT, PT, NCOL).transpose(1, 0, 2).reshape(PT, KT * NCOL)
        ).astype(BF16)
        # Slot 0's panel duplicated contiguously: [p, i*S + c].
        xt0_dev = np.ascontiguousarray(
            np.concatenate(
                [xt_dev[:, i * NCOL : i * NCOL + S] for i in range(KT)], axis=1
            )
        )
        # Per-core weight panel: 13 slots concatenated slot-major along
        # the column axis (wrap-around classes pad the last core).
        slot_cls = [(m * CPC + j) % C for j in range(CPC)]
        wt_m = np.ascontiguousarray(
            np.concatenate([W_t8[c] for c in slot_cls], axis=1)
        )
        in_maps.append({"xt": xt_dev, "xt0": xt0_dev, "wt": wt_m})

    res = run_bass_kernel_spmd(nc, in_maps, list(range(NCORES)))
    LAST_RESULTS = res

    out = np.empty((B, U), np.float32)
    for m in range(NCORES):
        panel = np.asarray(res.results[m]["out"]).astype(np.float32)
        for j in range(CPC):
            c = m * CPC + j
            if c >= C:
                continue
            r = rows_by_class[c]
            if r.size:
                out[r] = panel[S * j : S * j + r.size] + bias_np[c]
    return out


# revision 32
# speedup vs baseline: 1.1026x; 1.0279x over previous
"""Trainium2 kernel for per-class conditional dense (MoE-style routing).

    out[b] = x[b] @ W[classes[b]] + bias[classes[b]]
    x: [2048, 512] f32, classes: [2048, 1] int, W: [100, 512, 512] f32,
    bias: [100, 512] f32 -> out: [2048, 512] f32

Sharding: expert-parallel across 8 NeuronCores (grouped-GEMM style).
Class c is owned by core c // 13 (13 class slots per core). The host
routes each sample to the core owning its class, packing the samples of
each class into a fixed-width (S columns, zero-padded) block of a
transposed activation panel.

Precision strategy: the weight table is the dominant HBM traffic, so it
is stored as TRN fp8 E3M4 (float8e3, 4 mantissa bits) at 1 byte/elem --
half the bf16 bytes. W is pre-scaled by a power of two s so its range
fills E3M4's [0.25, 15.5] normal band, and x is divided by the same s
(exact in bf16), so out = (x/s) @ (sW) = x @ W with no epilogue fixup.
The PE consumes the fp8 weights directly as the moving operand against
a bf16 stationary x panel (mixed-dtype matmul, 1 cycle/row). Measured
end-to-end relative error ~1.5e-2 (E3M4 quantization of W dominates),
inside the 2e-2 gate.

With the stream halved the Tensor engine becomes the critical resource
(~26.6K moving columns ~= 11.1 us at 2.4 GHz), so the schedule keeps
the PE hot and the 16 SDMA engines at their per-packet service ceiling:

- Sync HWDGE ring: the weight chunks in slot order, ramping
  0.5/0.5/1/1/2/3/3/1/1 slots -- small first so the PE starts half a
  slot after first-byte, 6KB-per-partition lines mid-stream (~26
  GB/s/engine vs ~19 for 2KB lines), tapered at the end so the final
  slots' matmuls trail the last HBM byte minimally.
- Scalar HWDGE ring: slot 0's x panel duplicated into a tiny tensor
  (lands ~1us before the full panel, so the first matmuls are gated
  only by slot 0's weights), then the full x panel as one transfer.
- PE warm-up: the DVFS p-state reaches full clock only after ~3us of
  continuous matmul execution and resets on idle gaps (cold matmuls
  measure ~1.2GHz, half speed), so dummy matmuls on scratch bridge the
  DMA fill and the real matmuls continue the ramp seamlessly.
- Drains alternate Scalar-ACTIVATE / DVE so consecutive groups
  overlap; stores ride the Sync ring, whose FIFO defers their packets
  behind the remaining weight stream for free; the final slot is
  computed in U-halves on fresh PSUM tiles so its first drain+store
  overlap its second half's matmuls and the kernel-ending store is
  only 16 KB (post-stream stores are per-packet-service-bound, and the
  last store's HBM write receipt is on the critical path).

The host scatters the panel rows back to sample order and adds the
bias in fp32. Measured 26.6-29.4 us across runs (DVFS throttle and
thermal state dominate the spread) vs ~35 us for the bf16 baseline.
"""

import sys
import types

import numpy as np

try:
    import concourse.bass as bass
except ImportError:  # pragma: no cover - fallback for bare environments
    for _p in ("/opt/trn_rl_repo", "/root/.axon_site/_ro/trn_rl_repo"):
        if _p not in sys.path:
            sys.path.insert(0, _p)
    import concourse.bass as bass

try:  # pragma: no cover
    import antenv.axon_hooks  # noqa: F401
except ImportError:
    # bass_utils imports this when BASS_TRACE is set; the agent image's
    # antenv lacks it. Register a no-op shim so tracing degrades to a
    # plain (untraced) run instead of crashing.
    _hooks = types.ModuleType("antenv.axon_hooks")
    _hooks.get_axon_ntff_profile_hook = lambda: None
    _hooks.set_axon_ntff_profile_hook = lambda h: None
    sys.modules["antenv.axon_hooks"] = _hooks

import bass_rust
import ml_dtypes
import concourse.tile as tile
from concourse import mybir
from concourse.bass_utils import run_bass_kernel_spmd

B, D, U, C = 2048, 512, 512, 100
NCORES = 8
CPC = 13  # class slots per core (8 * 13 = 104 >= C)
PT = 128  # partition tile
KT = D // PT  # contraction-dim tiles
WSL = KT * U  # fp8 weight columns per slot
BF16 = ml_dtypes.bfloat16
FP8 = ml_dtypes.float8_e3m4
FP8_MAX = 15.5  # E3M4 max finite
N_WARM = 6  # 512-col no-dep PE warm-up matmuls (DVFS p-state ramp)
N_WARM_FINE = 4  # trailing 128-col warm-up matmuls (fine-grained bridge)
N_WARM_POST = 10  # 128-col xt0-gated matmuls bridging to the first chunk

# Weight-chunk ramp (in half-slot units): slot 0 arrives in k-halves
# so the PE's first matmuls start half a slot earlier, mid-stream
# chunks grow to 3 slots whose 6KB-per-partition lines sustain the
# full packet-service rate (~26 GB/s/engine measured vs ~19 for 2KB
# lines), then a taper so the final slots' matmuls trail the last HBM
# byte minimally.
CHUNK_HALVES = [1, 1, 2, 2, 4, 6, 6, 2, 2]
assert sum(CHUNK_HALVES) == 2 * CPC
HWSL = WSL // 2  # fp8 columns per half-slot

_PROG_CACHE = {}
LAST_RESULTS = None  # BassKernelResults of the most recent device run


def _split_multi_waits(nc):
    """Walrus on this image only accepts one sync wait per instruction.

    Tile emits multi-wait instructions (notably the kernel-tail Drain,
    which waits on every live semaphore). Split each extra wait onto a
    same-engine NoOp inserted immediately before the instruction.
    """
    for fn in nc.m.functions:
        for bb in fn.blocks:
            new = []
            changed = False
            for inst in bb.instructions:
                si = inst.sync_info
                waits = list(si.on_wait) if si else []
                if len(waits) > 1:
                    for idx, w in enumerate(waits[:-1]):
                        nop = mybir.InstNoOp(
                            name=f"{inst.name}-waitsplit{idx}", ins=[], outs=[]
                        )
                        nop.engine = inst.engine
                        nop.sync_info = bass_rust.SyncInfo(
                            on_wait=[w], on_update=[]
                        )
                        new.append(nop)
                    inst.sync_info = bass_rust.SyncInfo(
                        on_wait=[waits[-1]], on_update=list(si.on_update)
                    )
                    changed = True
                new.append(inst)
            if changed:
                bb.instructions = new


def _build_program(S):
    """One SPMD program, shared by all 8 cores; per-core data differs.

    Per core: xt [PT, KT*NCOL] bf16 (pre-tiled transposed class-blocked
    activations, pre-divided by the weight scale), wt [PT, CPC*WSL]
    float8e3 (pre-tiled, pre-scaled weight slots, slot-major columns)
    -> out [NCOL, U] bf16.
    """
    f32 = mybir.dt.float32
    bf16 = mybir.dt.bfloat16
    fp8 = mybir.dt.float8e3
    NCOL = CPC * S
    GRP = PT // S  # class slots sharing one PSUM bank / output tile
    OG = -(-CPC // GRP)  # output groups
    BR = GRP * S  # rows per PSUM bank / output tile

    nc = bass.Bass()
    xt = nc.dram_tensor("xt", [PT, KT * NCOL], bf16, kind="ExternalInput")
    # Slot 0's x panel, duplicated into its own tiny contiguous tensor
    # (32 KB): it lands ~1us before the full panel, so the PE's first
    # real matmuls are gated only by slot 0's weights.
    xt0 = nc.dram_tensor("xt0", [PT, KT * S], bf16, kind="ExternalInput")
    wt = nc.dram_tensor("wt", [PT, CPC * WSL], fp8, kind="ExternalInput")
    out = nc.dram_tensor("out", [NCOL, U], bf16, kind="ExternalOutput")

    with tile.TileContext(nc) as tc:
        with (
            tc.tile_pool(name="xp", bufs=1) as xp,
            tc.tile_pool(name="wp", bufs=1) as wp,
            tc.tile_pool(name="op", bufs=1) as op,
            tc.tile_pool(name="pp", bufs=3, space="PSUM") as pp,
            tc.tile_pool(name="pb", bufs=2, space="PSUM") as pbpool,
            tc.tile_pool(name="ap", bufs=1, space="PSUM") as apool,
        ):
            xt_t = xp.tile([PT, KT * NCOL], bf16, name="x")
            xt0_t = xp.tile([PT, KT * S], bf16, name="x0")
            w_t = wp.tile([PT, CPC * WSL], fp8, name="w")
            scr_s = xp.tile([PT, 512], bf16, name="scr")
            scr_p = apool.tile([PT, 512], f32, name="scrp")

            # Scalar HWDGE ring: slot-0 x panel first (tiny), then the
            # full panel as ONE transfer (3.3KB lines pack better than
            # two 1.6KB halves). Sync HWDGE ring: the weight chunks, in
            # slot order. Two rings share the 16 SDMA engines but
            # buffer independently.
            nc.scalar.dma_start(xt0_t[:], xt0[:, :])
            nc.scalar.dma_start(xt_t[:], xt[:, :])
            col = 0
            for ci, n in enumerate(CHUNK_HALVES):
                w = n * HWSL
                # The two big mid-stream chunks ride the Scalar ring,
                # which is idle once xt is out: both rings then drain
                # concurrently through the second half of the stream
                # (measured ~3.9us of PE stalls before the late chunks
                # with everything on one ring).
                q = nc.scalar if n >= 6 else nc.sync
                q.dma_start(w_t[:, col : col + w], wt[:, col : col + w])
                col += w

            # PE warm-up: the DVFS p-state needs ~3us of continuous
            # matmul execution to reach full clock, and any multi-us
            # idle gap resets it (measured: cold matmuls run at ~1.2GHz,
            # half speed). Free-running dummies on memset scratch
            # bridge the DMA fill; one dummy then absorbs the xt0 DMA
            # wait (LDWEIGHTS carries a single wait), and xt0-gated
            # 128-col dummies keep the PE hot until slot 0's weights
            # land.
            nc.gpsimd.memset(scr_s[:], 1.0)
            for _ in range(N_WARM):
                nc.tensor.matmul(
                    scr_p[:],
                    scr_s[:, :PT],
                    scr_s[:],
                    start=True,
                    stop=True,
                    skip_group_check=True,
                )
            for _ in range(N_WARM_FINE):
                nc.tensor.matmul(
                    scr_p[:, :PT],
                    scr_s[:, :PT],
                    scr_s[:, :PT],
                    start=True,
                    stop=True,
                    skip_group_check=True,
                )
            for _ in range(N_WARM_POST):
                nc.tensor.matmul(
                    scr_p[:, :PT],
                    xt0_t[:, :PT],
                    xt0_t[:, :PT],
                    start=True,
                    stop=True,
                    skip_group_check=True,
                )

            ots = [op.tile([BR, U], bf16, name=f"o{g}") for g in range(OG)]

            U2 = U // 2
            for j in range(CPC):
                g, r = divmod(j, GRP)
                last = j == CPC - 1 and r == 0
                if r == 0 and not last:
                    ps = pp.tile([BR, U], f32, tag="ps", name=f"ps{g}")
                rows = min(BR, NCOL - g * BR)
                if not last:
                    for i in range(KT):
                        if j == 0:
                            stat = xt0_t[:, i * S : (i + 1) * S]
                        else:
                            stat = xt_t[
                                :, i * NCOL + S * j : i * NCOL + S * (j + 1)
                            ]
                        nc.tensor.matmul(
                            ps[S * r : S * r + S, :],
                            stat,
                            w_t[:, (j * KT + i) * U : (j * KT + i + 1) * U],
                            start=(i == 0),
                            stop=(i == KT - 1),
                            # PE-array column offset = PSUM partition
                            # offset; auto-infer rejects some offsets,
                            # so pass it explicitly.
                            tile_position=(0, S * r),
                        )
                else:
                    # Final slot split in U-halves on fresh PSUM tiles
                    # (dep tracking is partition-granular; a shared
                    # tile would serialize half B behind half A's
                    # drain): half A's drain and store overlap half B's
                    # matmuls, and the kernel-ending store shrinks to
                    # 16 KB.
                    for uo in (0, U2):
                        psh = pbpool.tile([S, U2], f32, tag="psh", name=f"h{uo}")
                        for i in range(KT):
                            stat = xt_t[
                                :, i * NCOL + S * j : i * NCOL + S * (j + 1)
                            ]
                            nc.tensor.matmul(
                                psh[:, :],
                                stat,
                                w_t[
                                    :,
                                    (j * KT + i) * U + uo : (j * KT + i) * U
                                    + uo
                                    + U2,
                                ],
                                start=(i == 0),
                                stop=(i == KT - 1),
                                tile_position=(0, 0),
                            )
                        nc.vector.tensor_scalar_add(
                            ots[g][:rows, uo : uo + U2], psh[:rows, :], 0.0
                        )
                        q = nc.sync if uo == 0 else nc.scalar
                        q.dma_start(
                            out[g * BR : g * BR + rows, uo : uo + U2],
                            ots[g][:rows, uo : uo + U2],
                        )
                if j == 0:
                    # Fillers covering the gap between slot 0's matmuls
                    # and the arrival of the next weight chunk.
                    for _ in range(3):
                        nc.tensor.matmul(
                            scr_p[:, :PT],
                            xt0_t[:, :PT],
                            xt0_t[:, :PT],
                            start=True,
                            stop=True,
                            skip_group_check=True,
                        )
                if (r == GRP - 1 or j == CPC - 1) and not last:
                    # Drains alternate Scalar-ACTIVATE / DVE so
                    # consecutive groups' drains overlap at the tail.
                    if g % 2 == 0:
                        nc.scalar.copy(ots[g][:rows, :], ps[:rows, :])
                    else:
                        nc.vector.tensor_scalar_add(
                            ots[g][:rows, :], ps[:rows, :], 0.0
                        )
                    # Early stores ride the Sync ring: its FIFO
                    # naturally defers their packets behind the
                    # remaining weight stream (no mid-stream bandwidth
                    # theft). The second-to-last group's trigger goes
                    # on Scalar so the tail triggers issue concurrently
                    # (each costs ~0.6us).
                    q = nc.scalar if g == OG - 2 else nc.sync
                    q.dma_start(
                        out[g * BR : g * BR + rows, :], ots[g][:rows, :]
                    )
    _split_multi_waits(nc)
    return nc


def kernel(x, classes, kernel, bias):
    global LAST_RESULTS
    x = np.asarray(x, dtype=np.float32)
    W = np.asarray(kernel, dtype=np.float32)
    bias_np = np.asarray(bias, dtype=np.float32)
    cls = np.asarray(classes).reshape(-1).astype(np.int64)

    counts = np.bincount(cls, minlength=C)
    # Fixed column width per class slot; multiple of 8 for DMA alignment.
    S = int(max(32, -(-counts.max() // 8) * 8))
    if S not in _PROG_CACHE:
        _PROG_CACHE[S] = _build_program(S)
    nc = _PROG_CACHE[S]
    NCOL = CPC * S

    # Power-of-two weight scale filling E3M4's normal band; x carries
    # the inverse scale exactly (exponent shift), so out = x @ W.
    absmax = float(np.abs(W).max())
    s = float(2.0 ** np.floor(np.log2(FP8_MAX / absmax))) if absmax > 0 else 1.0

    order = np.argsort(cls, kind="stable")
    starts = np.zeros(C + 1, np.int64)
    np.cumsum(counts[:C], out=starts[1:])
    rows_by_class = [order[starts[c] : starts[c + 1]] for c in range(C)]

    # Weight slots, pre-tiled to the SBUF layout and cast to E3M4:
    # [c, p, i*U+u] holds s*W[c, i*128+p, u] so each slot's DMA line is
    # 2 KB contiguous per partition.
    W_t8 = (
        (W * s)
        .reshape(C, KT, PT, U)
        .transpose(0, 2, 1, 3)
        .reshape(C, PT, KT * U)
        .astype(FP8)
    )

    xs = x * np.float32(1.0 / s)
    in_maps = []
    for m in range(NCORES):
        xt_m = np.zeros((D, NCOL), np.float32)
        for j in range(CPC):
            c = m * CPC + j
            if c >= C:
                continue
            r = rows_by_class[c]
            if r.size:
                xt_m[:, S * j : S * j + r.size] = xs[r].T
        # Pre-tile x panel: [p, i*NCOL + c] = xt[i*128+p, c].
        xt_dev = np.ascontiguousarray(
            xt_m.reshape(KT, PT, NCOL).transpose(1, 0, 2).reshape(PT, KT * NCOL)
        ).astype(BF16)
        # Slot 0's panel duplicated contiguously: [p, i*S + c].
        xt0_dev = np.ascontiguousarray(
            np.concatenate(
                [xt_dev[:, i * NCOL : i * NCOL + S] for i in range(KT)], axis=1
            )
        )
        # Per-core weight panel: 13 slots concatenated slot-major along
        # the column axis (wrap-around classes pad the last core).
        slot_cls = [(m * CPC + j) % C for j in range(CPC)]
        wt_m = np.ascontiguousarray(
            np.concatenate([W_t8[c] for c in slot_cls], axis=1)
        )
        in_maps.append({"xt": xt_dev, "xt0": xt0_dev, "wt": wt_m})

    res = run_bass_kernel_spmd(nc, in_maps, list(range(NCORES)))
    LAST_RESULTS = res

    out = np.empty((B, U), np.float32)
    for m in range(NCORES):
        panel = np.asarray(res.results[m]["out"]).astype(np.float32)
        for j in range(CPC):
            c = m * CPC + j
            if c >= C:
                continue
            r = rows_by_class[c]
            if r.size:
                out[r] = panel[S * j : S * j + r.size] + bias_np[c]
    return out
